# revision 9
# baseline (speedup 1.0000x reference)
"""Trainium2 Bass kernel for nn_BaseEmbedLoss (segment-center cosine embedding loss).

Strategy (data-parallel over batch, 1 batch image per core x 8 cores):
  Single pass over the data, fp8 throughout the bandwidth-critical path:
    feats uploaded as fp8e4 [g, 32 dims | 1 | 0] (matmul-ready), onehot
    uploaded as fp8e4 [p, c, g] (built host-side from targets).
    per 128-pixel group g: matmul  acc += [f|1|0]_g^T @ [oh_g | oh_g*rinv_g]
    accumulated in PSUM across all groups -> [34, 40] with
      rows 0..31 = sums.T / nsum.T, row 32 = counts, col-blocks OH | OH*rinv.
    Consecutive groups ping-pong between PE column-halves (tile_position 0/64).
    rinv = 1/||f_pixel||: ACT square (fp8->bf16) + bf16 pairwise add tree
    (DVE 2x mode) + quake-rsqrt bit trick on the bf16 norm^2 (2 int DVE ops;
    ~2% error, which only perturbs seg_cos - a ~0.3% slice of the loss).
    oh*rinv built by one dense bf16->fp8-free DVE mult against a DMA-broadcast
    rinv (dense 2x mode; broadcast APs would force slow 1x mode).
  AllReduce of the [128, 40] accumulator (tiny) across 8 cores.
  Tiny C x C center-similarity stage computed redundantly on every core.

Key identity: seg_cos[c] = centers[c] . nsum[c] / cnorm[c], nsum[c] = sum_{n in c} f_n/|f_n|
so no second pass over the data is needed.
"""

import os
import sys

os.environ.setdefault("JAX_PLATFORMS", "axon")
sys.path.insert(0, "/opt/trn_rl_repo")

import numpy as np
import ml_dtypes

import concourse.bass as bass
import concourse.mybir as mybir
import concourse.bacc as bacc
import concourse.tile as tile
from concourse import bass_utils

F32 = mybir.dt.float32
BF16 = mybir.dt.bfloat16
FP8 = mybir.dt.float8e4
U16 = mybir.dt.uint16
AF = mybir.ActivationFunctionType
ALU = mybir.AluOpType
AX = mybir.AxisListType

# Problem shapes (hardcoded per contract)
B, D, H, W = 8, 32, 512, 512
C = 19
CP = 20          # classes padded to even width (class 19 is a dummy)
NCORES = 8
HWL = H * W      # 262144 pixels per core (batch-sharded)
PX = 128         # pixels per matmul group (partition/contraction dim)
G = int(os.environ.get("K_G", "128"))  # groups per supertile
ST = int(os.environ.get("K_ST", HWL // (PX * G)))  # supertiles
M = D + 2        # stationary cols: 32 dims + ones col + zero pad col
FEB = G * M      # fp8 feats bytes per partition per supertile
MAGIC = 0x5F37   # bf16 quake rsqrt magic (top 16 bits of 0x5f3759df)


def _kernel_body(nc, tc, feats, oh_d, ident, eye19, onesc, out_d, out_acc=None, dbg_d=None):
    env = os.environ
    rep = int(env.get("K_REP", "1"))
    single = bool(env.get("K_SINGLE"))

    with (
        tc.tile_pool(name="consts", bufs=1) as cpool,
        tc.tile_pool(name="fio", bufs=5) as fpool,
        tc.tile_pool(name="woh", bufs=4) as wpool,
        tc.tile_pool(name="sq", bufs=3) as sqpool,
        tc.tile_pool(name="tree", bufs=3) as tpool,
        tc.tile_pool(name="small", bufs=4) as spool,
        tc.tile_pool(name="fin", bufs=1) as finpool,
        tc.tile_pool(name="accps", bufs=1, space="PSUM") as acc_pool,
        tc.tile_pool(name="ps", bufs=1, space="PSUM") as ps_pool,
        tc.tile_pool(name="dram", bufs=1, space="DRAM") as dpool,
    ):
        # ---- constants ----
        ident_sb = cpool.tile([PX, PX], F32)
        nc.sync.dma_start(ident_sb[:], ident[:])
        eye_sb = cpool.tile([CP, CP], F32)
        nc.sync.dma_start(eye_sb[:], eye19[:])
        ones_sb = cpool.tile([CP, 1], F32)
        nc.sync.dma_start(ones_sb[:], onesc[:])

        # PSUM accumulator [128, 40]: even groups -> partitions 0..33
        # (tile_position (0,0)), odd groups -> partitions 64..97 ((0,64)).
        acc = acc_pool.tile([PX, 2 * CP], F32)
        accs = [acc[0:M, :], acc[64 : 64 + M, :]]
        acc3s = [a.rearrange("m (b c) -> m b c", b=2) for a in accs]

        for st_r in range(ST * rep):
            st = st_r % ST
            F = fpool.tile([PX, FEB], FP8, tag="F")
            nc.sync.dma_start(F[:], feats[st])
            Wt = wpool.tile([PX, 2 * CP * G], FP8, tag="Wt")
            W4 = Wt[:].rearrange("p (b c g) -> p b c g", b=2, c=CP)
            nc.sync.dma_start(Wt[:, 0 : CP * G], oh_d[st])
            F3 = F[:].rearrange("p (g m) -> p g m", g=G)

            # per-pixel 1/||f||: square (ACT, fp8 in -> bf16 out), bf16
            # pairwise add tree (DVE 2x), quake-rsqrt bit trick.
            SQ = sqpool.tile([PX, G * D], BF16, tag="SQ")
            SQ3 = SQ[:].rearrange("p (g d) -> p g d", g=G)
            nc.scalar.square(SQ3, F3[:, :, 0:D])
            T16 = tpool.tile([PX, G * 16], BF16, tag="T16")
            T16_3 = T16[:].rearrange("p (g d) -> p g d", g=G)
            nc.vector.tensor_add(T16_3, SQ3[:, :, 0:16], SQ3[:, :, 16:32])
            T8 = tpool.tile([PX, G * 8], BF16, tag="T8")
            T8_3 = T8[:].rearrange("p (g d) -> p g d", g=G)
            nc.vector.tensor_add(T8_3, T16_3[:, :, 0:8], T16_3[:, :, 8:16])
            T4 = tpool.tile([PX, G * 4], BF16, tag="T4")
            T4_3 = T4[:].rearrange("p (g d) -> p g d", g=G)
            nc.vector.tensor_add(T4_3, T8_3[:, :, 0:4], T8_3[:, :, 4:8])
            T2 = tpool.tile([PX, G * 2], BF16, tag="T2")
            T2_3 = T2[:].rearrange("p (g d) -> p g d", g=G)
            nc.vector.tensor_add(T2_3, T4_3[:, :, 0:2], T4_3[:, :, 2:4])
            NRM2 = tpool.tile([PX, G], BF16, tag="NRM2")
            NRM2_3 = NRM2[:].rearrange("p (g d) -> p g d", g=G)
            nc.vector.tensor_add(NRM2_3, T2_3[:, :, 0:1], T2_3[:, :, 1:2])

            # rinv = 1/sqrt(nrm2): ACT sqrt (bf16 -> f32) then the fast
            # custom-DVE reciprocal approximation (fp32-only), cast to fp8.
            NRM = tpool.tile([PX, G], F32, tag="NRM")
            nc.scalar.sqrt(NRM[:], NRM2[:])
            RQ = tpool.tile([PX, G], F32, tag="RQ")
            nc.vector.reciprocal_approx_fast(RQ[:], NRM[:])
            rinv8 = spool.tile([PX, G], FP8, tag="rinv8")
            with nc.allow_low_precision("rinv feeds fp8 matmul anyway"):
                nc.vector.tensor_copy(rinv8[:], RQ[:])

            # oh*rinv: DMA-broadcast rinv across the class axis (dense DVE
            # 2x beats broadcast-AP 1x), then one dense mult.
            R_exp = spool.tile([PX, CP * G], FP8, tag="R_exp")
            nc.gpsimd.dma_start(
                R_exp[:],
                rinv8[:]
                .rearrange("p (o g) -> p o g", o=1)
                .broadcast_to([PX, CP, G]),
            )
            R3 = R_exp[:].rearrange("p (c g) -> p c g", c=CP)
            nc.vector.tensor_tensor(W4[:, 1], W4[:, 0], R3, op=ALU.mult)

            if dbg_d is not None and st_r == 0:
                dbg = finpool.tile([PX, 4 * G], F32, name="dbgt")
                nc.vector.tensor_copy(dbg[:, 0:G], NRM2[:])
                nc.vector.tensor_copy(dbg[:, G : 2 * G], RQ[:])
                nc.vector.tensor_copy(dbg[:, 2 * G : 3 * G], rinv8[:])
                nc.vector.tensor_copy(dbg[:, 3 * G : 4 * G], R_exp[:, 0:G])
                nc.sync.dma_start(dbg_d[:], dbg[:])

            for g in range(G):
                half = g % 2
                nc.tensor.matmul(
                    acc3s[half],
                    F3[:, g, :],
                    W4[:, :, :, g],
                    start=(st_r == 0 and g < 2),
                    stop=(st_r == ST * rep - 1 and g >= G - 2),
                    tile_position=(0, 64 * half),
                )

        # ---- all-reduce the [128, 2*CP] accumulator ----
        acc_sb = finpool.tile([PX, 2 * CP], F32)
        nc.vector.memset(acc_sb[:], 0.0)
        nc.vector.tensor_copy(acc_sb[0:M, :], accs[0])
        nc.vector.tensor_copy(acc_sb[64 : 64 + M, :], accs[1])
        cc_in = dpool.tile([PX, 2 * CP], F32)
        cc_out = dpool.tile([PX, 2 * CP], F32)
        nc.gpsimd.dma_start(cc_in[:], acc_sb[:])
        if single:
            nc.gpsimd.dma_start(cc_out[:], cc_in[:])
        else:
            nc.gpsimd.collective_compute(
                "AllReduce",
                ALU.add,
                replica_groups=[list(range(NCORES))],
                ins=[cc_in[:].opt()],
                outs=[cc_out[:].opt()],
            )
        ar_sb = finpool.tile([PX, 2 * CP], F32)
        nc.gpsimd.dma_start(ar_sb[:], cc_out[:])
        if out_acc is not None:
            nc.sync.dma_start(out_acc[:], ar_sb[:])

        # ---- transpose to class-major; each OH/W2 block separately so both
        # land on partitions 0..CP-1, then fold the two tile-position halves ----
        tps = ps_pool.tile([CP, PX], F32, tag="tps")
        nc.tensor.transpose(tps[:], ar_sb[:, 0:CP], ident_sb[:])
        TAw = finpool.tile([CP, PX], F32)
        nc.vector.tensor_copy(TAw[:], tps[:])
        TA = finpool.tile([CP, M], F32)
        nc.vector.tensor_add(TA[:], TAw[:, 0:M], TAw[:, 64 : 64 + M])
        tps_b = ps_pool.tile([CP, PX], F32, tag="tps_b")
        nc.tensor.transpose(tps_b[:], ar_sb[:, CP : 2 * CP], ident_sb[:])
        TBw = finpool.tile([CP, PX], F32)
        nc.vector.tensor_copy(TBw[:], tps_b[:])
        TBn = finpool.tile([CP, M], F32)
        nc.vector.tensor_add(TBn[:], TBw[:, 0:M], TBw[:, 64 : 64 + M])

        counts = TA[0:CP, D : D + 1]
        sums = TA[0:CP, 0:D]
        nsum = TBn[0:CP, 0:D]

        def small(shape, tag, dt=F32):
            return finpool.tile(shape, dt, tag=tag, name=tag)

        denom = small([CP, 1], "denom")
        nc.vector.tensor_scalar_max(denom[:], counts, 1.0)
        rden = small([CP, 1], "rden")
        nc.vector.reciprocal(rden[:], denom[:])
        present = small([CP, 1], "present")
        nc.vector.tensor_scalar_min(present[:], counts, 1.0)

        centers = small([CP, D], "centers")
        nc.vector.tensor_scalar_mul(centers[:], sums, rden[:])

        csq = small([CP, D], "csq")
        cn2 = small([CP, 1], "cn2")
        nc.vector.tensor_mul(csq[:], centers[:], centers[:])
        nc.vector.reduce_sum(cn2[:], csq[:], axis=AX.X)
        cnorm = small([CP, 1], "cnorm")
        nc.scalar.sqrt(cnorm[:], cn2[:])
        cnc = small([CP, 1], "cnc")
        nc.vector.tensor_scalar_max(cnc[:], cnorm[:], 1e-30)
        rcn = small([CP, 1], "rcn")
        nc.vector.reciprocal(rcn[:], cnc[:])

        dotp = small([CP, D], "dotp")
        dotcn = small([CP, 1], "dotcn")
        nc.vector.tensor_mul(dotp[:], centers[:], nsum)
        nc.vector.reduce_sum(dotcn[:], dotp[:], axis=AX.X)
        mean_cos = small([CP, 1], "mean_cos")
        nc.vector.tensor_scalar(
            mean_cos[:], dotcn[:], rcn[:], rden[:], op0=ALU.mult, op1=ALU.mult
        )
        simc = small([CP, 1], "simc")
        nc.scalar.activation(simc[:], mean_cos[:], AF.Copy, bias=1.0, scale=-1.0)
        sim_contrib = small([CP, 1], "sim_contrib")
        nc.vector.tensor_mul(sim_contrib[:], simc[:], present[:])

        # cosM = (centers*rcn) @ (centers*rcn).T
        cs = small([CP, D], "cs")
        nc.vector.tensor_scalar_mul(cs[:], centers[:], rcn[:])
        tps2 = ps_pool.tile([D, CP], F32, tag="tps2")
        nc.tensor.transpose(tps2[:], cs[:], ident_sb[0:CP, 0:CP])
        cs_T = small([D, CP], "cs_T")
        nc.vector.tensor_copy(cs_T[:], tps2[:])
        cos_ps = ps_pool.tile([CP, CP], F32, tag="cos_ps")
        nc.tensor.matmul(cos_ps[:], cs_T[:], cs_T[:], start=True, stop=True)
        cosM = small([CP, CP], "cosM")
        nc.vector.tensor_copy(cosM[:], cos_ps[:])

        R = small([CP, CP], "R")
        nc.vector.tensor_relu(R[:], cosM[:])
        t1 = small([CP, CP], "t1")
        nc.scalar.activation(t1[:], cosM[:], AF.Copy, bias=1.0, scale=-1.0)
        A = small([CP, CP], "A")
        nc.vector.tensor_sub(A[:], t1[:], R[:])
        t2 = small([CP, CP], "t2")
        nc.vector.tensor_mul(t2[:], A[:], eye_sb[:])
        terms = small([CP, CP], "terms")
        nc.vector.tensor_add(terms[:], R[:], t2[:])
        rowsum = small([CP, 1], "rowsum")
        nc.vector.reduce_sum(rowsum[:], terms[:], axis=AX.X)
        diffc = small([CP, 1], "diffc")
        nc.scalar.mul(diffc[:], rowsum[:], 1.0 / C)
        diff_contrib = small([CP, 1], "diff_contrib")
        nc.vector.tensor_mul(diff_contrib[:], diffc[:], present[:])

        contrib = small([CP, 1], "contrib")
        nc.vector.tensor_add(contrib[:], sim_contrib[:], diff_contrib[:])
        fin_ps = ps_pool.tile([1, 1], F32, tag="fin_ps")
        nc.tensor.matmul(fin_ps[:], contrib[:], ones_sb[:], start=True, stop=True)
        fin_sb = small([1, 1], "fin_sb")
        nc.vector.tensor_copy(fin_sb[:], fin_ps[:])
        nc.sync.dma_start(out_d[:], fin_sb[:])


_CACHE = {}


def _build_nc():
    if "nc" in _CACHE:
        return _CACHE["nc"]
    ndev = 1 if os.environ.get("K_SINGLE") else NCORES
    nc = bacc.Bacc(
        "TRN2", target_bir_lowering=False, debug=False, num_devices=ndev
    )
    feats = nc.dram_tensor("feats", [ST, PX, FEB], FP8, kind="ExternalInput")
    oh_d = nc.dram_tensor("oh", [ST, PX, CP * G], FP8, kind="ExternalInput")
    ident = nc.dram_tensor("ident", [PX, PX], F32, kind="ExternalInput")
    eye19 = nc.dram_tensor("eye19", [CP, CP], F32, kind="ExternalInput")
    onesc = nc.dram_tensor("onesc", [CP, 1], F32, kind="ExternalInput")
    out_d = nc.dram_tensor("out", [1, 1], F32, kind="ExternalOutput")
    out_acc = None
    dbg_d = None
    if os.environ.get("K_DUMP"):
        out_acc = nc.dram_tensor("out_acc", [PX, 2 * CP], F32, kind="ExternalOutput")
    if os.environ.get("K_DBG"):
        dbg_d = nc.dram_tensor("dbg", [PX, 4 * G], F32, kind="ExternalOutput")
    with tile.TileContext(nc) as tc:
        _kernel_body(nc, tc, feats, oh_d, ident, eye19, onesc, out_d, out_acc, dbg_d)
    nc.compile()
    _CACHE["nc"] = nc
    return nc


F8NP = ml_dtypes.float8_e4m3


def _consts():
    if "consts" in _CACHE:
        return _CACHE["consts"]
    ident = np.eye(PX, dtype=np.float32)
    eye19 = np.eye(CP, dtype=np.float32)
    eye19[C, C] = 0.0  # dummy padded class contributes nothing
    onesc = np.ones((CP, 1), dtype=np.float32)
    eye_oh = np.eye(CP, dtype=np.float32).astype(F8NP)  # onehot lookup rows
    _CACHE["consts"] = (ident, eye19, onesc, eye_oh)
    return _CACHE["consts"]


def _shard_inputs(inputs, targets):
    """Host-side marshalling: batch-shard, cast to fp8, retile to the
    matmul-ready [supertile, pixel, (group, dim|1|0)] layout; expand the
    int labels to the fp8 one-hot moving-operand layout [p, c, g]."""
    inputs = np.asarray(inputs, dtype=np.float32)
    targets = np.asarray(targets)
    ident, eye19, onesc, eye_oh = _consts()
    in_maps = []
    for b in range(NCORES):
        # [D, H, W] -> [N, D] pixel-major (matches reference transpose/reshape)
        f = inputs[b].transpose(1, 2, 0).reshape(HWL, D)
        # -> [ST, PX, G, M] with ones/zero columns appended
        fb = np.zeros((ST, G, PX, M), dtype=F8NP)
        fb[:, :, :, 0:D] = f.reshape(ST, G, PX, D).astype(F8NP)
        fb[:, :, :, D] = np.float32(1.0)
        fb = np.ascontiguousarray(fb.transpose(0, 2, 1, 3)).reshape(ST, PX, FEB)
        lab = targets[b].reshape(ST, G, PX)
        oh = eye_oh[lab]  # [ST, G, PX, CP] fp8
        oh = np.ascontiguousarray(oh.transpose(0, 2, 3, 1)).reshape(
            ST, PX, CP * G
        )
        in_maps.append(
            {
                "feats": fb,
                "oh": oh,
                "ident": ident,
                "eye19": eye19,
                "onesc": onesc,
            }
        )
    return in_maps


def run_on_device(in_maps):
    nc = _build_nc()
    res = bass_utils.run_bass_kernel_spmd(
        nc, in_maps, core_ids=list(range(len(in_maps)))
    )
    return res


def kernel(inputs, targets, num_classes):
    assert int(num_classes) == C
    in_maps = _shard_inputs(inputs, targets)
    res = run_on_device(in_maps)
    out = np.asarray(res.results[0]["out"], dtype=np.float32).reshape(1)
    return out


if __name__ == "__main__":
    rng = np.random.default_rng(0)
    x = rng.standard_normal((B, D, H, W), dtype=np.float32)
    t = rng.integers(0, C, size=(B, H, W)).astype(np.int64)
    print(kernel(x, t, C))


# revision 11
# speedup vs baseline: 1.2763x; 1.2763x over previous
"""Trainium2 Bass kernel for nn_BaseEmbedLoss (segment-center cosine embedding loss).

Strategy (data-parallel over batch, 1 batch image per core x 8 cores):
  Single pass over the data, fp8 throughout the bandwidth-critical path:
    feats uploaded as fp8e4 [g, 32 dims | 1 | 0] (matmul-ready), onehot
    uploaded as fp8e4 [p, c, g] (built host-side from targets).
    per 128-pixel group g: matmul  acc += [f|1|0]_g^T @ [oh_g | oh_g*rinv_g]
    accumulated in PSUM across all groups -> [34, 40] with
      rows 0..31 = sums.T / nsum.T, row 32 = counts, col-blocks OH | OH*rinv.
    Consecutive groups ping-pong between PE column-halves (tile_position 0/64).
    rinv = 1/||f_pixel||: ACT square (fp8->bf16) + bf16 pairwise add tree
    (DVE 2x mode) + quake-rsqrt bit trick on the bf16 norm^2 (2 int DVE ops;
    ~2% error, which only perturbs seg_cos - a ~0.3% slice of the loss).
    oh*rinv built by one dense bf16->fp8-free DVE mult against a DMA-broadcast
    rinv (dense 2x mode; broadcast APs would force slow 1x mode).
  AllReduce of the [128, 40] accumulator (tiny) across 8 cores.
  Tiny C x C center-similarity stage computed redundantly on every core.

Key identity: seg_cos[c] = centers[c] . nsum[c] / cnorm[c], nsum[c] = sum_{n in c} f_n/|f_n|
so no second pass over the data is needed.
"""

import os
import sys

os.environ.setdefault("JAX_PLATFORMS", "axon")
sys.path.insert(0, "/opt/trn_rl_repo")

import numpy as np
import ml_dtypes

import concourse.bass as bass
import concourse.mybir as mybir
import concourse.bacc as bacc
import concourse.tile as tile
from concourse import bass_utils

F32 = mybir.dt.float32
BF16 = mybir.dt.bfloat16
FP8 = mybir.dt.float8e4
U16 = mybir.dt.uint16
AF = mybir.ActivationFunctionType
ALU = mybir.AluOpType
AX = mybir.AxisListType

# Problem shapes (hardcoded per contract)
B, D, H, W = 8, 32, 512, 512
C = 19
CP = 20          # classes padded to even width (class 19 is a dummy)
NCORES = 8
HWL = H * W      # 262144 pixels per core (batch-sharded)
PX = 128         # pixels per matmul group (partition/contraction dim)
G = int(os.environ.get("K_G", "128"))  # groups per supertile
ST = int(os.environ.get("K_ST", HWL // (PX * G)))  # supertiles
M = D + 2        # stationary cols: 32 dims + ones col + zero pad col
FEB = G * M      # fp8 feats bytes per partition per supertile
MAGIC = 0x5F37   # bf16 quake rsqrt magic (top 16 bits of 0x5f3759df)


def _kernel_body(nc, tc, feats, oh_d, ident, eye19, onesc, out_d, out_acc=None, dbg_d=None):
    env = os.environ
    rep = int(env.get("K_REP", "1"))
    single = bool(env.get("K_SINGLE"))

    with (
        tc.tile_pool(name="consts", bufs=1) as cpool,
        tc.tile_pool(name="fio", bufs=5) as fpool,
        tc.tile_pool(name="woh", bufs=4) as wpool,
        tc.tile_pool(name="sq", bufs=3) as sqpool,
        tc.tile_pool(name="tree", bufs=3) as tpool,
        tc.tile_pool(name="small", bufs=4) as spool,
        tc.tile_pool(name="fin", bufs=1) as finpool,
        tc.tile_pool(name="accps", bufs=1, space="PSUM") as acc_pool,
        tc.tile_pool(name="ps", bufs=1, space="PSUM") as ps_pool,
        tc.tile_pool(name="dram", bufs=1, space="DRAM") as dpool,
    ):
        # ---- constants ----
        ident_sb = cpool.tile([PX, PX], F32)
        nc.sync.dma_start(ident_sb[:], ident[:])
        eye_sb = cpool.tile([CP, CP], F32)
        nc.sync.dma_start(eye_sb[:], eye19[:])
        ones_sb = cpool.tile([CP, 1], F32)
        nc.sync.dma_start(ones_sb[:], onesc[:])

        # PSUM accumulator [128, 40]: even groups -> partitions 0..33
        # (tile_position (0,0)), odd groups -> partitions 64..97 ((0,64)).
        acc = acc_pool.tile([PX, 2 * CP], F32)
        accs = [acc[0:M, :], acc[64 : 64 + M, :]]
        acc3s = [a.rearrange("m (b c) -> m b c", b=2) for a in accs]

        for st_r in range(ST * rep):
            st = st_r % ST
            F = fpool.tile([PX, FEB], FP8, tag="F")
            nc.sync.dma_start(F[:], feats[st])
            Wt = wpool.tile([PX, 2 * CP * G], BF16, tag="Wt")
            W4 = Wt[:].rearrange("p (b c g) -> p b c g", b=2, c=CP)
            nc.scalar.dma_start(Wt[:, 0 : CP * G], oh_d[st])
            F3 = F[:].rearrange("p (g m) -> p g m", g=G)

            # per-pixel 1/||f||: square (ACT, fp8 in -> bf16 out), bf16
            # pairwise add tree (DVE 2x), quake-rsqrt bit trick.
            SQ = sqpool.tile([PX, G * D], BF16, tag="SQ")
            SQ3 = SQ[:].rearrange("p (g d) -> p g d", g=G)
            nc.scalar.square(SQ3, F3[:, :, 0:D])
            T16 = tpool.tile([PX, G * 16], BF16, tag="T16")
            T16_3 = T16[:].rearrange("p (g d) -> p g d", g=G)
            nc.vector.tensor_add(T16_3, SQ3[:, :, 0:16], SQ3[:, :, 16:32])
            T8 = tpool.tile([PX, G * 8], BF16, tag="T8")
            T8_3 = T8[:].rearrange("p (g d) -> p g d", g=G)
            nc.vector.tensor_add(T8_3, T16_3[:, :, 0:8], T16_3[:, :, 8:16])
            T4 = tpool.tile([PX, G * 4], BF16, tag="T4")
            T4_3 = T4[:].rearrange("p (g d) -> p g d", g=G)
            nc.vector.tensor_add(T4_3, T8_3[:, :, 0:4], T8_3[:, :, 4:8])
            T2 = tpool.tile([PX, G * 2], BF16, tag="T2")
            T2_3 = T2[:].rearrange("p (g d) -> p g d", g=G)
            nc.vector.tensor_add(T2_3, T4_3[:, :, 0:2], T4_3[:, :, 2:4])
            NRM2 = tpool.tile([PX, G], BF16, tag="NRM2")
            NRM2_3 = NRM2[:].rearrange("p (g d) -> p g d", g=G)
            nc.vector.tensor_add(NRM2_3, T2_3[:, :, 0:1], T2_3[:, :, 1:2])

            # rinv = 1/sqrt(nrm2): ACT sqrt (bf16 -> f32) then the fast
            # custom-DVE reciprocal approximation (fp32-only), cast to fp8.
            NRM = tpool.tile([PX, G], F32, tag="NRM")
            nc.scalar.sqrt(NRM[:], NRM2[:])
            RQ = tpool.tile([PX, G], F32, tag="RQ")
            nc.vector.reciprocal_approx_fast(RQ[:], NRM[:])
            rinv8 = spool.tile([PX, G], BF16, tag="rinv8")
            with nc.allow_low_precision("rinv feeds bf16 matmul anyway"):
                nc.vector.tensor_copy(rinv8[:], RQ[:])

            # oh*rinv: DMA-broadcast rinv across the class axis (dense DVE
            # 2x beats broadcast-AP 1x), then one dense mult.
            R_exp = spool.tile([PX, CP * G], BF16, tag="R_exp")
            nc.gpsimd.dma_start(
                R_exp[:],
                rinv8[:]
                .rearrange("p (o g) -> p o g", o=1)
                .broadcast_to([PX, CP, G]),
            )
            R3 = R_exp[:].rearrange("p (c g) -> p c g", c=CP)
            nc.vector.tensor_tensor(W4[:, 1], W4[:, 0], R3, op=ALU.mult)

            if dbg_d is not None and st_r == 0:
                dbg = finpool.tile([PX, 4 * G], F32, name="dbgt")
                nc.vector.tensor_copy(dbg[:, 0:G], NRM2[:])
                nc.vector.tensor_copy(dbg[:, G : 2 * G], RQ[:])
                nc.vector.tensor_copy(dbg[:, 2 * G : 3 * G], rinv8[:])
                nc.vector.tensor_copy(dbg[:, 3 * G : 4 * G], R_exp[:, 0:G])
                nc.sync.dma_start(dbg_d[:], dbg[:])

            for g in range(G):
                half = g % 2
                nc.tensor.matmul(
                    acc3s[half],
                    F3[:, g, :],
                    W4[:, :, :, g],
                    start=(st_r == 0 and g < 2),
                    stop=(st_r == ST * rep - 1 and g >= G - 2),
                    tile_position=(0, 64 * half),
                )

        # ---- fold tile-position halves, then all-reduce the [M, 2*CP] acc ----
        acc_h0 = finpool.tile([M, 2 * CP], F32)
        nc.vector.tensor_copy(acc_h0[:], accs[0])
        acc_sb = finpool.tile([M, 2 * CP], F32)
        nc.vector.tensor_add(acc_sb[:], acc_h0[:], accs[1])
        cc_in = dpool.tile([M, 2 * CP], F32)
        cc_out = dpool.tile([M, 2 * CP], F32)
        nc.sync.dma_start(cc_in[:], acc_sb[:])
        if single:
            nc.gpsimd.dma_start(cc_out[:], cc_in[:])
        else:
            nc.gpsimd.collective_compute(
                "AllReduce",
                ALU.add,
                replica_groups=[list(range(NCORES))],
                ins=[cc_in[:].opt()],
                outs=[cc_out[:].opt()],
            )
        ar_sb = finpool.tile([M, 2 * CP], F32)
        nc.sync.dma_start(ar_sb[:], cc_out[:])
        if out_acc is not None:
            nc.sync.dma_start(out_acc[:], ar_sb[:])

        # ---- transpose to class-major; each OH/W2 block separately so both
        # land on partitions 0..CP-1, then fold the two tile-position halves ----
        tps = ps_pool.tile([CP, M], F32, tag="tps")
        nc.tensor.transpose(tps[:], ar_sb[:, 0:CP], ident_sb[0:M, 0:M])
        TA = finpool.tile([CP, M], F32)
        nc.vector.tensor_copy(TA[:], tps[:])
        tps_b = ps_pool.tile([CP, M], F32, tag="tps_b")
        nc.tensor.transpose(tps_b[:], ar_sb[:, CP : 2 * CP], ident_sb[0:M, 0:M])
        TBn = finpool.tile([CP, M], F32)
        nc.vector.tensor_copy(TBn[:], tps_b[:])

        counts = TA[0:CP, D : D + 1]
        sums = TA[0:CP, 0:D]
        nsum = TBn[0:CP, 0:D]

        def small(shape, tag, dt=F32):
            return finpool.tile(shape, dt, tag=tag, name=tag)

        denom = small([CP, 1], "denom")
        nc.vector.tensor_scalar_max(denom[:], counts, 1.0)
        rden = small([CP, 1], "rden")
        nc.vector.reciprocal(rden[:], denom[:])
        present = small([CP, 1], "present")
        nc.vector.tensor_scalar_min(present[:], counts, 1.0)

        centers = small([CP, D], "centers")
        nc.vector.tensor_scalar_mul(centers[:], sums, rden[:])

        csq = small([CP, D], "csq")
        cn2 = small([CP, 1], "cn2")
        nc.vector.tensor_mul(csq[:], centers[:], centers[:])
        nc.vector.reduce_sum(cn2[:], csq[:], axis=AX.X)
        cnorm = small([CP, 1], "cnorm")
        nc.scalar.sqrt(cnorm[:], cn2[:])
        cnc = small([CP, 1], "cnc")
        nc.vector.tensor_scalar_max(cnc[:], cnorm[:], 1e-30)
        rcn = small([CP, 1], "rcn")
        nc.vector.reciprocal(rcn[:], cnc[:])

        dotp = small([CP, D], "dotp")
        dotcn = small([CP, 1], "dotcn")
        nc.vector.tensor_mul(dotp[:], centers[:], nsum)
        nc.vector.reduce_sum(dotcn[:], dotp[:], axis=AX.X)
        mean_cos = small([CP, 1], "mean_cos")
        nc.vector.tensor_scalar(
            mean_cos[:], dotcn[:], rcn[:], rden[:], op0=ALU.mult, op1=ALU.mult
        )
        simc = small([CP, 1], "simc")
        nc.scalar.activation(simc[:], mean_cos[:], AF.Copy, bias=1.0, scale=-1.0)
        sim_contrib = small([CP, 1], "sim_contrib")
        nc.vector.tensor_mul(sim_contrib[:], simc[:], present[:])

        # cosM = (centers*rcn) @ (centers*rcn).T
        cs = small([CP, D], "cs")
        nc.vector.tensor_scalar_mul(cs[:], centers[:], rcn[:])
        tps2 = ps_pool.tile([D, CP], F32, tag="tps2")
        nc.tensor.transpose(tps2[:], cs[:], ident_sb[0:CP, 0:CP])
        cs_T = small([D, CP], "cs_T")
        nc.vector.tensor_copy(cs_T[:], tps2[:])
        cos_ps = ps_pool.tile([CP, CP], F32, tag="cos_ps")
        nc.tensor.matmul(cos_ps[:], cs_T[:], cs_T[:], start=True, stop=True)
        cosM = small([CP, CP], "cosM")
        nc.vector.tensor_copy(cosM[:], cos_ps[:])

        R = small([CP, CP], "R")
        nc.vector.tensor_relu(R[:], cosM[:])
        t1 = small([CP, CP], "t1")
        nc.scalar.activation(t1[:], cosM[:], AF.Copy, bias=1.0, scale=-1.0)
        A = small([CP, CP], "A")
        nc.vector.tensor_sub(A[:], t1[:], R[:])
        t2 = small([CP, CP], "t2")
        nc.vector.tensor_mul(t2[:], A[:], eye_sb[:])
        terms = small([CP, CP], "terms")
        nc.vector.tensor_add(terms[:], R[:], t2[:])
        rowsum = small([CP, 1], "rowsum")
        nc.vector.reduce_sum(rowsum[:], terms[:], axis=AX.X)
        diffc = small([CP, 1], "diffc")
        nc.scalar.mul(diffc[:], rowsum[:], 1.0 / C)
        diff_contrib = small([CP, 1], "diff_contrib")
        nc.vector.tensor_mul(diff_contrib[:], diffc[:], present[:])

        contrib = small([CP, 1], "contrib")
        nc.vector.tensor_add(contrib[:], sim_contrib[:], diff_contrib[:])
        fin_ps = ps_pool.tile([1, 1], F32, tag="fin_ps")
        nc.tensor.matmul(fin_ps[:], contrib[:], ones_sb[:], start=True, stop=True)
        fin_sb = small([1, 1], "fin_sb")
        nc.vector.tensor_copy(fin_sb[:], fin_ps[:])
        nc.sync.dma_start(out_d[:], fin_sb[:])


_CACHE = {}


def _build_nc():
    if "nc" in _CACHE:
        return _CACHE["nc"]
    ndev = 1 if os.environ.get("K_SINGLE") else NCORES
    nc = bacc.Bacc(
        "TRN2", target_bir_lowering=False, debug=False, num_devices=ndev
    )
    feats = nc.dram_tensor("feats", [ST, PX, FEB], FP8, kind="ExternalInput")
    oh_d = nc.dram_tensor("oh", [ST, PX, CP * G], BF16, kind="ExternalInput")
    ident = nc.dram_tensor("ident", [PX, PX], F32, kind="ExternalInput")
    eye19 = nc.dram_tensor("eye19", [CP, CP], F32, kind="ExternalInput")
    onesc = nc.dram_tensor("onesc", [CP, 1], F32, kind="ExternalInput")
    out_d = nc.dram_tensor("out", [1, 1], F32, kind="ExternalOutput")
    out_acc = None
    dbg_d = None
    if os.environ.get("K_DUMP"):
        out_acc = nc.dram_tensor("out_acc", [M, 2 * CP], F32, kind="ExternalOutput")
    if os.environ.get("K_DBG"):
        dbg_d = nc.dram_tensor("dbg", [PX, 4 * G], F32, kind="ExternalOutput")
    with tile.TileContext(nc) as tc:
        _kernel_body(nc, tc, feats, oh_d, ident, eye19, onesc, out_d, out_acc, dbg_d)
    nc.compile()
    _CACHE["nc"] = nc
    return nc


F8NP = ml_dtypes.float8_e4m3


def _consts():
    if "consts" in _CACHE:
        return _CACHE["consts"]
    ident = np.eye(PX, dtype=np.float32)
    eye19 = np.eye(CP, dtype=np.float32)
    eye19[C, C] = 0.0  # dummy padded class contributes nothing
    onesc = np.ones((CP, 1), dtype=np.float32)
    eye_oh = np.eye(CP, dtype=np.float32).astype(ml_dtypes.bfloat16)  # onehot lookup rows
    _CACHE["consts"] = (ident, eye19, onesc, eye_oh)
    return _CACHE["consts"]


def _shard_inputs(inputs, targets):
    """Host-side marshalling: batch-shard, cast to fp8, retile to the
    matmul-ready [supertile, pixel, (group, dim|1|0)] layout; expand the
    int labels to the fp8 one-hot moving-operand layout [p, c, g]."""
    inputs = np.asarray(inputs, dtype=np.float32)
    targets = np.asarray(targets)
    ident, eye19, onesc, eye_oh = _consts()
    in_maps = []
    for b in range(NCORES):
        # [D, H, W] -> [N, D] pixel-major (matches reference transpose/reshape)
        f = inputs[b].transpose(1, 2, 0).reshape(HWL, D)
        # -> [ST, PX, G, M] with ones/zero columns appended
        fb = np.zeros((ST, G, PX, M), dtype=F8NP)
        fb[:, :, :, 0:D] = f.reshape(ST, G, PX, D).astype(F8NP)
        fb[:, :, :, D] = np.float32(1.0)
        fb = np.ascontiguousarray(fb.transpose(0, 2, 1, 3)).reshape(ST, PX, FEB)
        lab = targets[b].reshape(ST, G, PX)
        oh = eye_oh[lab]  # [ST, G, PX, CP] fp8
        oh = np.ascontiguousarray(oh.transpose(0, 2, 3, 1)).reshape(
            ST, PX, CP * G
        )
        in_maps.append(
            {
                "feats": fb,
                "oh": oh,
                "ident": ident,
                "eye19": eye19,
                "onesc": onesc,
            }
        )
    return in_maps


def run_on_device(in_maps):
    nc = _build_nc()
    res = bass_utils.run_bass_kernel_spmd(
        nc, in_maps, core_ids=list(range(len(in_maps)))
    )
    return res


def kernel(inputs, targets, num_classes):
    assert int(num_classes) == C
    in_maps = _shard_inputs(inputs, targets)
    res = run_on_device(in_maps)
    out = np.asarray(res.results[0]["out"], dtype=np.float32).reshape(1)
    return out


if __name__ == "__main__":
    rng = np.random.default_rng(0)
    x = rng.standard_normal((B, D, H, W), dtype=np.float32)
    t = rng.integers(0, C, size=(B, H, W)).astype(np.int64)
    print(kernel(x, t, C))
